# revision 8
# baseline (speedup 1.0000x reference)
"""Trainium2 Bass kernel for MultiHeadAttention with RoPE (cross-attention).

Reference computation (B=4, C=512, T=S=2048, H=8 heads, dc=64):
    q = Wq x + bq ; k = Wk c + bk ; v = Wv c + bv        (1x1 convs)
    q,k <- RoPE(q,k)
    out = softmax(q k^T / 8) v                            (per head)
    y = Wo out + bo

Sharding: 8 cores = (batch b in 0..3) x (T-half j in 0..1).  Core (b,j)
computes the full output slice y[b, :, j*1024:(j+1)*1024] and needs only
x[b,:,tslice] plus all of c[b] (k/v recomputed on both cores of a batch).
No collectives; the host reassembles the 8 disjoint output slices.

Device-side algebraic tricks:
  * 1/sqrt(dc) folded into Wq host-side.
  * RoPE(q) = q*cos + (R q)*sin with R the fixed rotate-half matrix, so the
    rotated stream is just a second projection with weights R@Wq — all matmul.
  * Biases enter via an augmented contraction row (x_hat has a ones row,
    W_hat has a bias row), so projections are exact for nonzero biases.
  * scores are computed TRANSPOSED ([s, t] layout) so that softmax(p) can be
    consumed directly by the AV matmul without any transpose.
  * V is projected in transposed layout [s, o] with one extra all-ones column
    per head; the AV matmul then yields the softmax denominator for free.
  * Denominator broadcast across partitions via a tiny K=2 selector matmul.
Matmuls run as float32r (full-rate fp32 PE mode).
"""

import math
import os
from contextlib import ExitStack

import numpy as np

import concourse.bass as bass
import concourse.tile as tile
from concourse import bacc, mybir
from concourse import bass_utils

F32 = mybir.dt.float32
F32R = mybir.dt.float32r
AF = mybir.ActivationFunctionType

# Problem constants (hardcoded per contract)
B, C, T = 4, 512, 2048
H, DC = 8, 64
THETA = 10000.0
NCORES = 8
TH = T // 2            # 1024: per-core t slice
P = 128                # partitions
NT = 512               # matmul free-dim tile
KC = C // P            # 4 contraction chunks over channels
OC = C // P            # 4 output-channel chunks
SCN = T // P           # 16 s chunks
VW = H * (DC + 1)      # 520: vT width (64 chans + 1 ones col per head)

USE_F32R = os.environ.get("KERNEL_F32R", "1") == "1"


# --------------------------------------------------------------------------
# device program
# --------------------------------------------------------------------------

def _emit(tc, d, y, use_f32r):
    nc = tc.nc
    MD = F32R if use_f32r else F32  # dtype for matmul-feeding tensors
    mm = lambda ap: ap

    with ExitStack() as ctx:
        # persistent pools (live to end of kernel)
        p_wo = ctx.enter_context(tc.tile_pool(name="p_wo", bufs=1))
        p_ms = ctx.enter_context(tc.tile_pool(name="p_ms", bufs=1))
        p_q = ctx.enter_context(tc.tile_pool(name="p_q", bufs=1))
        p_k = ctx.enter_context(tc.tile_pool(name="p_k", bufs=1))
        p_vt = ctx.enter_context(tc.tile_pool(name="p_vt", bufs=1))

        wo_ch = []
        for i in range(KC):
            t_ = p_wo.tile([P, C], MD, name=f"wo{i}", tag=f"wo{i}")
            nc.sync.dma_start(t_[:], d["woT"][i * P:(i + 1) * P, :])
            wo_ch.append(t_)
        bo4 = p_ms.tile([P, OC], F32, name="bo4", tag="bo4")
        nc.sync.dma_start(bo4[:], d["bo4"][:])
        sel = p_ms.tile([1, P], F32, name="sel", tag="sel")
        nc.sync.dma_start(sel[:], d["sel"][:])

        q_sb = [p_q.tile([P, TH], MD, name=f"q{i}", tag=f"q{i}") for i in range(OC)]
        k_sb = [p_k.tile([P, T], MD, name=f"k{i}", tag=f"k{i}") for i in range(OC)]
        vt_sb = [p_vt.tile([P, VW], MD, name=f"vt{i}", tag=f"vt{i}") for i in range(SCN)]

        def load_w(pool, dname, width, brow):
            """Load 4 [P,width] weight chunks; bias row goes to `brow` [1,width]."""
            chunks = []
            for i in range(KC):
                t_ = pool.tile([P, width], MD, name=f"{dname}{i}", tag=f"{dname}{i}")
                nc.sync.dma_start(t_[:], d[dname][i * P:(i + 1) * P, :])
                chunks.append(t_)
            nc.sync.dma_start(brow[0:1, :], d[dname][C:C + 1, :])
            chunks.append(brow)  # chunks[KC] is the bias row
            return chunks

        def proj_psum(pp, w_ch, in_ch, oc, fslice, n):
            """Accumulate one [P, n] projection tile into PSUM (5 chunks)."""
            ps = pp.tile([P, n], F32, name="ps", tag="ps")
            for kc in range(KC):
                nc.tensor.matmul(
                    ps[:], mm(w_ch[kc][:, oc * P:(oc + 1) * P]),
                    mm(in_ch[kc][:, fslice]), start=(kc == 0), stop=False)
            nc.tensor.matmul(
                ps[:], mm(w_ch[KC][:, oc * P:(oc + 1) * P]),
                mm(in_ch[KC][:, fslice]), start=False, stop=True)
            return ps

        with tc.tile_pool(name="pc", bufs=1) as pc:
            c_ch = []
            for i in range(KC):
                t_ = pc.tile([P, T], MD, name=f"c{i}", tag=f"c{i}")
                nc.sync.dma_start(t_[:], d["ch"][i * P:(i + 1) * P, :])
                c_ch.append(t_)
            cb = pc.tile([1, T], MD, name="cb", tag="cb")
            nc.sync.dma_start(cb[:], d["ch"][C:C + 1, :])
            c_ch.append(cb)

            # ---- Q projection + RoPE ----
            with tc.tile_pool(name="pwq", bufs=1) as pwq, \
                 tc.tile_pool(name="px", bufs=1) as px, \
                 tc.tile_pool(name="pprj", space="PSUM", bufs=4) as pprj:
                bq1 = pwq.tile([1, C], MD, name="bq1", tag="bq1")
                bqr1 = pwq.tile([1, C], MD, name="bqr1", tag="bqr1")
                wq_ch = load_w(pwq, "wqT", C, bq1)
                wqr_ch = load_w(pwq, "wqrT", C, bqr1)
                x_ch = []
                for i in range(KC):
                    t_ = px.tile([P, TH], MD, name=f"x{i}", tag=f"x{i}")
                    nc.sync.dma_start(t_[:], d["xh"][i * P:(i + 1) * P, :])
                    x_ch.append(t_)
                xb = px.tile([1, TH], MD, name="xb", tag="xb")
                nc.sync.dma_start(xb[:], d["xh"][C:C + 1, :])
                x_ch.append(xb)
                cosq = px.tile([P, TH], F32, name="cosq", tag="cosq")
                nc.sync.dma_start(cosq[:], d["cosq"][:])
                sinq = px.tile([P, TH], F32, name="sinq", tag="sinq")
                nc.sync.dma_start(sinq[:], d["sinq"][:])
                rtmp_pool = px  # scratch for rope partial products

                for oc in range(OC):
                    for tb in range(TH // NT):
                        fs = slice(tb * NT, (tb + 1) * NT)
                        ps_q = proj_psum(pprj, wq_ch, x_ch, oc, fs, NT)
                        ps_qr = proj_psum(pprj, wqr_ch, x_ch, oc, fs, NT)
                        t1 = rtmp_pool.tile([P, NT], F32, name="rt1", tag="rt1", bufs=2)
                        nc.vector.tensor_mul(t1[:], ps_q[:], cosq[:, fs])
                        t2 = rtmp_pool.tile([P, NT], F32, name="rt2", tag="rt2", bufs=2)
                        nc.vector.tensor_mul(t2[:], ps_qr[:], sinq[:, fs])
                        nc.vector.tensor_add(q_sb[oc][:, fs], t1[:], t2[:])

            # ---- K projection + RoPE ----
            with tc.tile_pool(name="pwk", bufs=1) as pwk, \
                 tc.tile_pool(name="ptk", bufs=1) as ptk, \
                 tc.tile_pool(name="pprk", space="PSUM", bufs=4) as pprk:
                bk1 = pwk.tile([1, C], MD, name="bk1", tag="bk1")
                bkr1 = pwk.tile([1, C], MD, name="bkr1", tag="bkr1")
                wk_ch = load_w(pwk, "wkT", C, bk1)
                wkr_ch = load_w(pwk, "wkrT", C, bkr1)
                cosk = ptk.tile([P, T], F32, name="cosk", tag="cosk")
                nc.sync.dma_start(cosk[:], d["cosk"][:])
                sink = ptk.tile([P, T], F32, name="sink", tag="sink")
                nc.sync.dma_start(sink[:], d["sink"][:])

                for oc in range(OC):
                    for sb in range(T // NT):
                        fs = slice(sb * NT, (sb + 1) * NT)
                        ps_k = proj_psum(pprk, wk_ch, c_ch, oc, fs, NT)
                        ps_kr = proj_psum(pprk, wkr_ch, c_ch, oc, fs, NT)
                        t1 = ptk.tile([P, NT], F32, name="kt1", tag="kt1", bufs=2)
                        nc.vector.tensor_mul(t1[:], ps_k[:], cosk[:, fs])
                        t2 = ptk.tile([P, NT], F32, name="kt2", tag="kt2", bufs=2)
                        nc.vector.tensor_mul(t2[:], ps_kr[:], sink[:, fs])
                        nc.vector.tensor_add(k_sb[oc][:, fs], t1[:], t2[:])

            # ---- V projection (transposed layout [s, o] + ones cols) ----
            with tc.tile_pool(name="pwv", bufs=1) as pwv, \
                 tc.tile_pool(name="pprv", space="PSUM", bufs=4) as pprv:
                bv1 = pwv.tile([1, VW], MD, name="bv1", tag="bv1")
                wv_ch = load_w(pwv, "wvT", VW, bv1[0:1, :])
                HALF = VW // 2  # 260
                for sc in range(SCN):
                    for hf in range(2):
                        cs = slice(hf * HALF, (hf + 1) * HALF)
                        pv = pprv.tile([P, HALF], F32, name="pv", tag="pv")
                        for kc in range(KC):
                            nc.tensor.matmul(
                                pv[:], mm(c_ch[kc][:, sc * P:(sc + 1) * P]),
                                mm(wv_ch[kc][:, cs]), start=(kc == 0), stop=False)
                        nc.tensor.matmul(
                            pv[:], mm(c_ch[KC][:, sc * P:(sc + 1) * P]),
                            mm(wv_ch[KC][:, cs]), start=False, stop=True)
                        nc.scalar.copy(vt_sb[sc][:, cs], pv[:])

        # ---- attention: scoresT -> exp -> AV (+denominator) -> normalize ----
        p_att = ctx.enter_context(tc.tile_pool(name="p_att", bufs=1))
        att_sb = [p_att.tile([P, TH], MD, name=f"att{i}", tag=f"att{i}")
                  for i in range(OC)]
        with tc.tile_pool(name="ppsc", space="PSUM", bufs=2) as ppsc, \
             tc.tile_pool(name="ppav", space="PSUM", bufs=3) as ppav, \
             tc.tile_pool(name="ppbc", space="PSUM", bufs=2) as ppbc, \
             tc.tile_pool(name="ppt", bufs=3) as ppt, \
             tc.tile_pool(name="pden", bufs=2) as pden:
            for m in range(OC):          # head pair (heads 2m, 2m+1)
                for tb in range(TH // NT):
                    fs = slice(tb * NT, (tb + 1) * NT)
                    avs, dens = [], []
                    for hp in range(2):
                        h = 2 * m + hp
                        rows = slice(hp * DC, (hp + 1) * DC)
                        av = ppav.tile([DC + 1, NT], F32, name="av", tag="av")
                        for sc in range(SCN):
                            sp = ppsc.tile([P, NT], F32, name="sp", tag="sp")
                            nc.tensor.matmul(
                                sp[:], mm(k_sb[m][rows, sc * P:(sc + 1) * P]),
                                mm(q_sb[m][rows, fs]), start=True, stop=True)
                            pt = ppt.tile([P, NT], MD, name="pt", tag="pt")
                            nc.scalar.activation(pt[:], sp[:], AF.Exp)
                            nc.tensor.matmul(
                                av[:], mm(vt_sb[sc][:, h * (DC + 1):(h + 1) * (DC + 1)]),
                                mm(pt[:]), start=(sc == 0), stop=(sc == SCN - 1))
                        dn = pden.tile([1, NT], F32, name="dn", tag="dn", bufs=4)
                        nc.vector.reciprocal(dn[0:1, :], av[DC:DC + 1, :])
                        avs.append(av)
                        dens.append(dn)
                    bc = ppbc.tile([P, NT], F32, name="bc", tag="bc")
                    for hp in range(2):
                        orows = slice(hp * DC, (hp + 1) * DC)
                        nc.tensor.matmul(bc[orows, :], mm(sel[0:1, 0:DC]),
                                         mm(dens[hp][0:1, :]),
                                         start=True, stop=True)
                    bcs = pden.tile([P, NT], F32, name="bcs", tag="bcs", bufs=2)
                    nc.vector.tensor_copy(bcs[:], bc[:])
                    for hp in range(2):
                        orows = slice(hp * DC, (hp + 1) * DC)
                        nc.vector.tensor_mul(
                            att_sb[m][orows, fs], avs[hp][0:DC, :], bcs[orows, :])

        # ---- output projection + bias ----
        with tc.tile_pool(name="ppy", space="PSUM", bufs=2) as ppy, \
             tc.tile_pool(name="pys", bufs=2) as pys:
            for tb in range(TH // NT):
                fs = slice(tb * NT, (tb + 1) * NT)
                for of in range(OC):
                    yp = ppy.tile([P, NT], F32, name="yp", tag="yp")
                    for oc in range(KC):
                        nc.tensor.matmul(
                            yp[:], mm(wo_ch[oc][:, of * P:(of + 1) * P]),
                            mm(att_sb[oc][:, fs]),
                            start=(oc == 0), stop=(oc == KC - 1))
                    ys = pys.tile([P, NT], F32, name="ys", tag="ys")
                    nc.vector.tensor_scalar_add(ys[:], yp[:], bo4[:, of:of + 1])
                    nc.sync.dma_start(y[of * P:(of + 1) * P, fs], ys[:])


def build_program(use_f32r=USE_F32R):
    nc = bacc.Bacc("TRN2", target_bir_lowering=False, debug=False,
                   num_devices=NCORES)
    d = {}
    MD = F32R if use_f32r else F32

    def din(name, shape, dt=F32):
        d[name] = nc.dram_tensor(name, shape, dt, kind="ExternalInput").ap()

    din("xh", [C + 1, TH], MD)
    din("ch", [C + 1, T], MD)
    din("wqT", [C + 1, C], MD)
    din("wqrT", [C + 1, C], MD)
    din("wkT", [C + 1, C], MD)
    din("wkrT", [C + 1, C], MD)
    din("wvT", [C + 1, VW], MD)
    din("woT", [C, C], MD)
    din("bo4", [P, OC])
    din("cosq", [P, TH])
    din("sinq", [P, TH])
    din("cosk", [P, T])
    din("sink", [P, T])
    din("sel", [1, P])
    y = nc.dram_tensor("y", [C, TH], F32, kind="ExternalOutput").ap()

    with tile.TileContext(nc) as tc:
        _emit(tc, d, y, use_f32r)
    nc.compile()
    return nc


# --------------------------------------------------------------------------
# host-side input prep / output assembly
# --------------------------------------------------------------------------

def _rot_matrix():
    """R such that (R q)[i] matches reference rotate-half per 64-chan head."""
    R = np.zeros((C, C), np.float32)
    half = DC // 2
    for h in range(H):
        b0 = h * DC
        for i in range(half):
            R[b0 + i, b0 + half + i] = -1.0
            R[b0 + half + i, b0 + i] = 1.0
    return R


def _rope_tables():
    inv = 1.0 / (THETA ** (np.arange(0, DC, 2, dtype=np.float32) / DC))  # [32]
    f = np.arange(T, dtype=np.float32)[:, None] * inv[None, :]           # [T,32]
    pos = np.concatenate([f, f], axis=-1)                                # [T,64]
    cos_t, sin_t = np.cos(pos), np.sin(pos)                              # [T,64]
    # [128, T]: row r covers head-pair channel r, channel dim = r % 64
    cos_tab = np.ascontiguousarray(np.tile(cos_t.T, (2, 1)), np.float32)
    sin_tab = np.ascontiguousarray(np.tile(sin_t.T, (2, 1)), np.float32)
    return cos_tab, sin_tab


def _aug(Wt, bias):
    """[C+1, n] array: W^T stacked with the bias row."""
    return np.ascontiguousarray(
        np.concatenate([Wt, bias[None, :]], axis=0), np.float32)


def make_in_maps(x, c, Wq, bq, Wk, bk, Wv, bv, Wo, bo):
    scale = 1.0 / math.sqrt(DC)
    R = _rot_matrix()
    Wq_s, bq_s = Wq * scale, bq * scale
    wqT = _aug(Wq_s.T, bq_s)
    wqrT = _aug((R @ Wq_s).T, R @ bq_s)
    wkT = _aug(Wk.T, bk)
    wkrT = _aug((R @ Wk).T, R @ bk)

    # V^T with a ones column appended per head (cols h*65+64)
    wvT = np.zeros((C + 1, VW), np.float32)
    for h in range(H):
        wvT[:C, h * (DC + 1):h * (DC + 1) + DC] = Wv[h * DC:(h + 1) * DC, :].T
        wvT[C, h * (DC + 1):h * (DC + 1) + DC] = bv[h * DC:(h + 1) * DC]
        wvT[C, h * (DC + 1) + DC] = 1.0  # ones column via the bias row

    woT = np.ascontiguousarray(Wo.T, np.float32)
    bo4 = np.ascontiguousarray(bo.reshape(OC, P).T, np.float32)

    cos_tab, sin_tab = _rope_tables()
    selm = np.ones((1, P), np.float32)

    ones_t = np.ones((1, T), np.float32)
    in_maps = []
    for core in range(NCORES):
        b, j = core // 2, core % 2
        ts = slice(j * TH, (j + 1) * TH)
        xh = np.concatenate([x[b][:, ts], ones_t[:, :TH]], axis=0)
        ch = np.concatenate([c[b], ones_t], axis=0)
        in_maps.append({
            "xh": np.ascontiguousarray(xh, np.float32),
            "ch": np.ascontiguousarray(ch, np.float32),
            "wqT": wqT, "wqrT": wqrT, "wkT": wkT, "wkrT": wkrT,
            "wvT": wvT, "woT": woT, "bo4": bo4,
            "cosq": np.ascontiguousarray(cos_tab[:, ts]),
            "sinq": np.ascontiguousarray(sin_tab[:, ts]),
            "cosk": cos_tab, "sink": sin_tab,
            "sel": selm,
        })
    return in_maps


def assemble(results):
    Y = np.empty((B, C, T), np.float32)
    for core in range(NCORES):
        b, j = core // 2, core % 2
        Y[b, :, j * TH:(j + 1) * TH] = results[core]["y"]
    return Y


_CACHE = {}


def _get_program():
    key = USE_F32R
    if key not in _CACHE:
        _CACHE[key] = build_program(key)
    return _CACHE[key]


def run(trace=False, **inputs):
    nc = _get_program()
    in_maps = make_in_maps(**inputs)
    res = bass_utils.run_bass_kernel_spmd(
        nc, in_maps, core_ids=list(range(NCORES)), trace=trace)
    return assemble(res.results), res


def kernel(**inputs):
    out, _ = run(trace=False, **inputs)
    return out


# revision 15
# speedup vs baseline: 1.2213x; 1.2213x over previous
"""Trainium2 Bass kernel for MultiHeadAttention with RoPE (cross-attention).

Reference computation (B=4, C=512, T=S=2048, H=8 heads, dc=64):
    q = Wq x + bq ; k = Wk c + bk ; v = Wv c + bv        (1x1 convs)
    q,k <- RoPE(q,k)
    out = softmax(q k^T / 8) v                            (per head)
    y = Wo out + bo

Sharding: 8 cores = (batch b in 0..3) x (T-half j in 0..1).  Core (b,j)
computes the full output slice y[b, :, j*1024:(j+1)*1024] and needs only
x[b,:,tslice] plus all of c[b] (k/v recomputed on both cores of a batch).
No collectives; the host reassembles the 8 disjoint output slices.

Device-side structure:
  * 1/sqrt(dc) folded into Wq host-side.
  * RoPE(q) = q*cos + (R q)*sin with R the fixed rotate-half matrix, so the
    rotated stream is just a second projection with weights R@Wq — all matmul.
  * When any bias is nonzero, biases enter via an augmented contraction row
    (input gains a ones row, W gains a bias row); with all-zero biases (the
    setup_inputs case) the extra chunk is skipped entirely.
  * scores are computed TRANSPOSED ([s, t] layout) so that softmax(p) can be
    consumed directly by the AV matmul without any transpose.
  * V is projected in transposed layout [s, o] with one extra column per
    head that is forced to 1.0; the AV matmul then yields the softmax
    denominator for free.
  * softmax denominator: reciprocal_approx_fast (DVE) + gpsimd
    partition_broadcast, then one DVE multiply per head/t-block.
  * All heavy matmuls run as float32r; consecutive matmuls share a stationary
    operand (two moving blocks per weight load) to halve LDWEIGHTS traffic.
  * fp32r matmuls do not register as PE activity for the HAM clock gate, so
    the PE gets throttled to 1.2 GHz mid-kernel; tiny bf16 "keep-alive"
    matmuls are sprinkled through every phase to hold the clock at 2.4 GHz.
"""

import math
import os
from contextlib import ExitStack

import numpy as np

import concourse.bass as bass
import concourse.tile as tile
from concourse import bacc, mybir
from concourse import bass_utils

F32 = mybir.dt.float32
F32R = mybir.dt.float32r
BF16 = mybir.dt.bfloat16
AF = mybir.ActivationFunctionType

# Problem constants (hardcoded per contract)
B, C, T = 4, 512, 2048
H, DC = 8, 64
THETA = 10000.0
NCORES = 8
TH = T // 2            # 1024: per-core t slice
P = 128                # partitions
NT = 512               # matmul free-dim tile
KC = C // P            # 4 contraction chunks over channels
OC = C // P            # 4 output-channel chunks
SCN = T // P           # 16 s chunks
VW = H * (DC + 1)      # 520: vT width (64 chans + 1 ones col per head)
VH = VW // 2           # 260: half-width for one-bank psum tiles

USE_F32R = os.environ.get("KERNEL_F32R", "1") == "1"
USE_KEEPALIVE = os.environ.get("KERNEL_KEEPALIVE", "1") == "1"


# --------------------------------------------------------------------------
# device program
# --------------------------------------------------------------------------

def _emit(tc, d, y, use_f32r, use_bias):
    nc = tc.nc
    MD = F32R if use_f32r else F32  # dtype for matmul-feeding tensors
    NK = KC + 1 if use_bias else KC  # contraction chunks incl. optional bias

    with ExitStack() as ctx:
        # persistent pools (live to end of kernel)
        p_wo = ctx.enter_context(tc.tile_pool(name="p_wo", bufs=1))
        p_ms = ctx.enter_context(tc.tile_pool(name="p_ms", bufs=1))
        p_q = ctx.enter_context(tc.tile_pool(name="p_q", bufs=1))
        p_k = ctx.enter_context(tc.tile_pool(name="p_k", bufs=1))
        p_vt = ctx.enter_context(tc.tile_pool(name="p_vt", bufs=1))

        wo_ch = []
        for i in range(KC):
            t_ = p_wo.tile([P, C], MD, name=f"wo{i}", tag=f"wo{i}")
            nc.sync.dma_start(t_[:], d["woT"][i * P:(i + 1) * P, :])
            wo_ch.append(t_)
        bo4 = p_ms.tile([P, OC], F32, name="bo4", tag="bo4")
        nc.sync.dma_start(bo4[:], d["bo4"][:])

        # keep-alive: tiny bf16 matmul that counts as PE activity for HAM
        ka = p_ms.tile([1, DC], BF16, name="ka", tag="ka")
        nc.vector.memset(ka[:], 0.03125)

        def keepalive(pp):
            if not USE_KEEPALIVE:
                return
            kt = pp.tile([DC, DC], F32, name="kt", tag="ka", bufs=1)
            nc.tensor.matmul(kt[0:DC, 0:DC], ka[0:1, :], ka[0:1, :],
                             start=True, stop=True)

        q_sb = [p_q.tile([P, TH], MD, name=f"q{i}", tag=f"q{i}") for i in range(OC)]
        k_sb = [p_k.tile([P, T], MD, name=f"k{i}", tag=f"k{i}") for i in range(OC)]
        vt_sb = [p_vt.tile([P, VW], MD, name=f"vt{i}", tag=f"vt{i}") for i in range(SCN)]

        def load_w(pool, dname, width, brow):
            chunks = []
            for i in range(KC):
                t_ = pool.tile([P, width], MD, name=f"{dname}{i}", tag=f"{dname}{i}")
                nc.sync.dma_start(t_[:], d[dname][i * P:(i + 1) * P, :])
                chunks.append(t_)
            if use_bias:
                nc.sync.dma_start(brow[0:1, :], d[dname][C:C + 1, :])
                chunks.append(brow)  # chunks[KC] is the bias row
            return chunks

        with tc.tile_pool(name="pc", bufs=1) as pc:
            # c always carries a ones row: the V projection uses it to build
            # the per-head ones columns (softmax denominator) and, with
            # use_bias, the bias contributions.
            c_ch = []
            for i in range(KC):
                t_ = pc.tile([P, T], MD, name=f"c{i}", tag=f"c{i}")
                nc.sync.dma_start(t_[:], d["ch"][i * P:(i + 1) * P, :])
                c_ch.append(t_)
            cb = pc.tile([1, T], MD, name="cb", tag="cb")
            nc.sync.dma_start(cb[:], d["ch"][C:C + 1, :])
            c_ch.append(cb)

            # ---- Q projection + RoPE ----
            with tc.tile_pool(name="pwq", bufs=1) as pwq, \
                 tc.tile_pool(name="px", bufs=1) as px, \
                 tc.tile_pool(name="pprj", space="PSUM", bufs=6) as pprj:
                bq1 = pwq.tile([1, C], MD, name="bq1", tag="bq1")
                bqr1 = pwq.tile([1, C], MD, name="bqr1", tag="bqr1")
                wq_ch = load_w(pwq, "wqT", C, bq1)
                wqr_ch = load_w(pwq, "wqrT", C, bqr1)
                x_ch = []
                for i in range(KC):
                    t_ = px.tile([P, TH], MD, name=f"x{i}", tag=f"x{i}")
                    nc.sync.dma_start(t_[:], d["xh"][i * P:(i + 1) * P, :])
                    x_ch.append(t_)
                if use_bias:
                    xb = px.tile([1, TH], MD, name="xb", tag="xb")
                    nc.sync.dma_start(xb[:], d["xh"][C:C + 1, :])
                    x_ch.append(xb)
                cosq = px.tile([P, TH], F32, name="cosq", tag="cosq")
                nc.sync.dma_start(cosq[:], d["cosq"][:])
                sinq = px.tile([P, TH], F32, name="sinq", tag="sinq")
                nc.sync.dma_start(sinq[:], d["sinq"][:])

                for oc in range(OC):
                    ocs = slice(oc * P, (oc + 1) * P)
                    keepalive(pprj)
                    psq, psqr = [], []
                    for tb in range(2):
                        psq.append(pprj.tile([P, NT], F32, name="psq", tag="ps"))
                        psqr.append(pprj.tile([P, NT], F32, name="psqr", tag="ps"))
                    for kc in range(NK):
                        st, sp = (kc == 0), (kc == NK - 1)
                        for tb in range(2):  # same stationary, two t-blocks
                            fs = slice(tb * NT, (tb + 1) * NT)
                            nc.tensor.matmul(psq[tb][:], wq_ch[kc][:, ocs],
                                             x_ch[kc][:, fs], start=st, stop=sp)
                        for tb in range(2):
                            fs = slice(tb * NT, (tb + 1) * NT)
                            nc.tensor.matmul(psqr[tb][:], wqr_ch[kc][:, ocs],
                                             x_ch[kc][:, fs], start=st, stop=sp)
                    for tb in range(2):
                        fs = slice(tb * NT, (tb + 1) * NT)
                        t1 = px.tile([P, NT], F32, name="rt1", tag="rt1", bufs=2)
                        nc.vector.tensor_mul(t1[:], psq[tb][:], cosq[:, fs])
                        t2 = px.tile([P, NT], F32, name="rt2", tag="rt2", bufs=2)
                        nc.vector.tensor_mul(t2[:], psqr[tb][:], sinq[:, fs])
                        nc.vector.tensor_add(q_sb[oc][:, fs], t1[:], t2[:])

            # ---- K projection + RoPE ----
            with tc.tile_pool(name="pwk", bufs=1) as pwk, \
                 tc.tile_pool(name="ptk", bufs=1) as ptk, \
                 tc.tile_pool(name="pprk", space="PSUM", bufs=6) as pprk:
                bk1 = pwk.tile([1, C], MD, name="bk1", tag="bk1")
                bkr1 = pwk.tile([1, C], MD, name="bkr1", tag="bkr1")
                wk_ch = load_w(pwk, "wkT", C, bk1)
                wkr_ch = load_w(pwk, "wkrT", C, bkr1)
                cosk = ptk.tile([P, T], F32, name="cosk", tag="cosk")
                nc.sync.dma_start(cosk[:], d["cosk"][:])
                sink = ptk.tile([P, T], F32, name="sink", tag="sink")
                nc.sync.dma_start(sink[:], d["sink"][:])

                for oc in range(OC):
                    ocs = slice(oc * P, (oc + 1) * P)
                    for sbp in range(2):  # s-block pairs
                        keepalive(pprk)
                        psk, pskr = [], []
                        for i in range(2):
                            psk.append(pprk.tile([P, NT], F32, name="psk", tag="ps"))
                            pskr.append(pprk.tile([P, NT], F32, name="pskr", tag="ps"))
                        for kc in range(NK):
                            st, sp = (kc == 0), (kc == NK - 1)
                            for i in range(2):
                                fs = slice((2 * sbp + i) * NT, (2 * sbp + i + 1) * NT)
                                nc.tensor.matmul(psk[i][:], wk_ch[kc][:, ocs],
                                                 c_ch[kc][:, fs], start=st, stop=sp)
                            for i in range(2):
                                fs = slice((2 * sbp + i) * NT, (2 * sbp + i + 1) * NT)
                                nc.tensor.matmul(pskr[i][:], wkr_ch[kc][:, ocs],
                                                 c_ch[kc][:, fs], start=st, stop=sp)
                        for i in range(2):
                            fs = slice((2 * sbp + i) * NT, (2 * sbp + i + 1) * NT)
                            t1 = ptk.tile([P, NT], F32, name="kt1", tag="kt1", bufs=2)
                            nc.vector.tensor_mul(t1[:], psk[i][:], cosk[:, fs])
                            t2 = ptk.tile([P, NT], F32, name="kt2", tag="kt2", bufs=2)
                            nc.vector.tensor_mul(t2[:], pskr[i][:], sink[:, fs])
                            nc.vector.tensor_add(k_sb[oc][:, fs], t1[:], t2[:])

            # ---- V projection (transposed layout [s, o] + ones cols) ----
            with tc.tile_pool(name="pwv", bufs=1) as pwv, \
                 tc.tile_pool(name="pprv", space="PSUM", bufs=4) as pprv:
                # V projection always runs the augmented KC+1 chunks: the
                # wvT bias row carries the per-head ones columns (and bias).
                bv1 = pwv.tile([1, VW], MD, name="bv1", tag="bv1")
                wv_ch = [None] * (KC + 1)
                for i in range(KC):
                    wv_ch[i] = pwv.tile([P, VW], MD, name=f"wv{i}", tag=f"wv{i}")
                    nc.sync.dma_start(wv_ch[i][:], d["wvT"][i * P:(i + 1) * P, :])
                nc.sync.dma_start(bv1[0:1, :], d["wvT"][C:C + 1, :])
                wv_ch[KC] = bv1
                for sc in range(SCN):
                    scs = slice(sc * P, (sc + 1) * P)
                    if sc % 4 == 0:
                        keepalive(pprv)
                    pv = [pprv.tile([P, VH], F32, name="pv", tag="pv")
                          for _ in range(2)]
                    for kc in range(KC + 1):
                        st, sp = (kc == 0), (kc == KC)
                        for hf in range(2):  # same stationary, two col halves
                            cs = slice(hf * VH, (hf + 1) * VH)
                            nc.tensor.matmul(pv[hf][:], c_ch[kc][:, scs],
                                             wv_ch[kc][:, cs], start=st, stop=sp)
                    for hf in range(2):
                        cs = slice(hf * VH, (hf + 1) * VH)
                        nc.vector.tensor_copy(vt_sb[sc][:, cs], pv[hf][:])

        # ---- attention: scoresT -> exp -> AV (+denominator) -> normalize ----
        p_att = ctx.enter_context(tc.tile_pool(name="p_att", bufs=1))
        att_sb = [p_att.tile([P, TH], MD, name=f"att{i}", tag=f"att{i}")
                  for i in range(OC)]
        with tc.tile_pool(name="ppsc", space="PSUM", bufs=3) as ppsc, \
             tc.tile_pool(name="ppav", space="PSUM", bufs=4) as ppav, \
             tc.tile_pool(name="ppt", bufs=4) as ppt, \
             tc.tile_pool(name="pden", bufs=2) as pden:
            for m in range(OC):          # head pair (heads 2m, 2m+1)
                for hp in range(2):
                    h = 2 * m + hp
                    rows = slice(hp * DC, (hp + 1) * DC)
                    hs = slice(h * (DC + 1), (h + 1) * (DC + 1))
                    av = [ppav.tile([DC + 1, NT], F32, name="av", tag="av")
                          for _ in range(2)]
                    for sc in range(SCN):
                        scs = slice(sc * P, (sc + 1) * P)
                        if sc % 4 == 0:
                            keepalive(ppsc)
                        sps = []
                        for tb in range(2):  # same k-chunk stationary
                            fs = slice(tb * NT, (tb + 1) * NT)
                            sp = ppsc.tile([P, NT], F32, name="sp", tag="sp")
                            nc.tensor.matmul(sp[:], k_sb[m][rows, scs],
                                             q_sb[m][rows, fs],
                                             start=True, stop=True)
                            sps.append(sp)
                        pts = []
                        for tb in range(2):
                            pt = ppt.tile([P, NT], MD, name="pt", tag="pt")
                            nc.scalar.activation(pt[:], sps[tb][:], AF.Exp)
                            pts.append(pt)
                        for tb in range(2):  # same vT stationary
                            nc.tensor.matmul(av[tb][:], vt_sb[sc][:, hs],
                                             pts[tb][:], start=(sc == 0),
                                             stop=(sc == SCN - 1))
                    for tb in range(2):
                        fs = slice(tb * NT, (tb + 1) * NT)
                        dr = pden.tile([1, NT], F32, name="dr", tag="dr", bufs=4)
                        nc.vector.tensor_copy(dr[0:1, :], av[tb][DC:DC + 1, :])
                        dn = pden.tile([1, NT], F32, name="dn", tag="dn", bufs=4)
                        nc.vector.reciprocal_approx_fast(dn[0:1, :], dr[0:1, :])
                        bcs = pden.tile([DC, NT], F32, name="bcs", tag="bcs",
                                        bufs=2)
                        nc.gpsimd.partition_broadcast(bcs[:], dn[0:1, :])
                        nc.vector.tensor_mul(att_sb[m][rows, fs],
                                             av[tb][0:DC, :], bcs[:])

        if d.get("dbg_q") is not None:
            for i in range(OC):
                nc.sync.dma_start(d["dbg_q"][i * P:(i + 1) * P, :],
                                  q_sb[i][:].bitcast(F32))
                nc.sync.dma_start(d["dbg_k"][i * P:(i + 1) * P, :],
                                  k_sb[i][:].bitcast(F32))
                nc.sync.dma_start(d["dbg_att"][i * P:(i + 1) * P, :],
                                  att_sb[i][:].bitcast(F32))
            for i in range(SCN):
                nc.sync.dma_start(d["dbg_vt"][i * P:(i + 1) * P, :],
                                  vt_sb[i][:].bitcast(F32))

        # ---- output projection + bias ----
        with tc.tile_pool(name="ppy", space="PSUM", bufs=4) as ppy, \
             tc.tile_pool(name="pys", bufs=4) as pys:
            for of in range(OC):
                ofs = slice(of * P, (of + 1) * P)
                keepalive(ppy)
                yp = [ppy.tile([P, NT], F32, name="yp", tag="yp")
                      for _ in range(2)]
                for oc in range(KC):
                    st, sp = (oc == 0), (oc == KC - 1)
                    for tb in range(2):  # same Wo stationary
                        fs = slice(tb * NT, (tb + 1) * NT)
                        nc.tensor.matmul(yp[tb][:], wo_ch[oc][:, ofs],
                                         att_sb[oc][:, fs], start=st, stop=sp)
                for tb in range(2):
                    fs = slice(tb * NT, (tb + 1) * NT)
                    ys = pys.tile([P, NT], F32, name="ys", tag="ys")
                    nc.vector.tensor_scalar_add(ys[:], yp[tb][:],
                                                bo4[:, of:of + 1])
                    nc.sync.dma_start(y[ofs, fs], ys[:])


def build_program(use_f32r=USE_F32R, use_bias=False):
    nc = bacc.Bacc("TRN2", target_bir_lowering=False, debug=False,
                   num_devices=NCORES)
    d = {}
    MD = F32R if use_f32r else F32
    CE = C + 1 if use_bias else C

    def din(name, shape, dt=F32):
        d[name] = nc.dram_tensor(name, shape, dt, kind="ExternalInput").ap()

    din("xh", [CE, TH], MD)
    din("ch", [C + 1, T], MD)
    din("wqT", [CE, C], MD)
    din("wqrT", [CE, C], MD)
    din("wkT", [CE, C], MD)
    din("wkrT", [CE, C], MD)
    din("wvT", [C + 1, VW], MD)
    din("woT", [C, C], MD)
    din("bo4", [P, OC])
    din("cosq", [P, TH])
    din("sinq", [P, TH])
    din("cosk", [P, T])
    din("sink", [P, T])
    y = nc.dram_tensor("y", [C, TH], F32, kind="ExternalOutput").ap()
    if os.environ.get("KERNEL_DEBUG_DUMPS") == "1":
        d["dbg_q"] = nc.dram_tensor("dbg_q", [C, TH], F32, kind="ExternalOutput").ap()
        d["dbg_k"] = nc.dram_tensor("dbg_k", [C, T], F32, kind="ExternalOutput").ap()
        d["dbg_att"] = nc.dram_tensor("dbg_att", [C, TH], F32, kind="ExternalOutput").ap()
        d["dbg_vt"] = nc.dram_tensor("dbg_vt", [T, VW], F32, kind="ExternalOutput").ap()
    else:
        d["dbg_q"] = None

    with tile.TileContext(nc) as tc:
        _emit(tc, d, y, use_f32r, use_bias)
    nc.compile()
    return nc


# --------------------------------------------------------------------------
# host-side input prep / output assembly
# --------------------------------------------------------------------------

def _rot_matrix():
    """R such that (R q)[i] matches reference rotate-half per 64-chan head."""
    R = np.zeros((C, C), np.float32)
    half = DC // 2
    for h in range(H):
        b0 = h * DC
        for i in range(half):
            R[b0 + i, b0 + half + i] = -1.0
            R[b0 + half + i, b0 + i] = 1.0
    return R


def _rope_tables():
    inv = 1.0 / (THETA ** (np.arange(0, DC, 2, dtype=np.float32) / DC))  # [32]
    f = np.arange(T, dtype=np.float32)[:, None] * inv[None, :]           # [T,32]
    pos = np.concatenate([f, f], axis=-1)                                # [T,64]
    cos_t, sin_t = np.cos(pos), np.sin(pos)                              # [T,64]
    # [128, T]: row r covers head-pair channel r, channel dim = r % 64
    cos_tab = np.ascontiguousarray(np.tile(cos_t.T, (2, 1)), np.float32)
    sin_tab = np.ascontiguousarray(np.tile(sin_t.T, (2, 1)), np.float32)
    return cos_tab, sin_tab


def make_in_maps(x, c, Wq, bq, Wk, bk, Wv, bv, Wo, bo, use_bias):
    scale = 1.0 / math.sqrt(DC)
    R = _rot_matrix()
    Wq_s, bq_s = Wq * scale, bq * scale

    def aug(Wt, bias):
        if not use_bias:
            return np.ascontiguousarray(Wt, np.float32)
        return np.ascontiguousarray(
            np.concatenate([Wt, bias[None, :]], axis=0), np.float32)

    wqT = aug(Wq_s.T, bq_s)
    wqrT = aug((R @ Wq_s).T, R @ bq_s)
    wkT = aug(Wk.T, bk)
    wkrT = aug((R @ Wk).T, R @ bk)

    # V^T always augmented: bias row carries per-head ones cols (h*65+64)
    wvT = np.zeros((C + 1, VW), np.float32)
    for h in range(H):
        wvT[:C, h * (DC + 1):h * (DC + 1) + DC] = Wv[h * DC:(h + 1) * DC, :].T
        wvT[C, h * (DC + 1):h * (DC + 1) + DC] = bv[h * DC:(h + 1) * DC]
        wvT[C, h * (DC + 1) + DC] = 1.0  # ones column via the bias row

    woT = np.ascontiguousarray(Wo.T, np.float32)
    bo4 = np.ascontiguousarray(bo.reshape(OC, P).T, np.float32)

    cos_tab, sin_tab = _rope_tables()

    ones_t = np.ones((1, T), np.float32)
    in_maps = []
    for core in range(NCORES):
        b, j = core // 2, core % 2
        ts = slice(j * TH, (j + 1) * TH)
        if use_bias:
            xh = np.concatenate([x[b][:, ts], ones_t[:, :TH]], axis=0)
        else:
            xh = x[b][:, ts]
        ch = np.concatenate([c[b], ones_t], axis=0)
        in_maps.append({
            "xh": np.ascontiguousarray(xh, np.float32),
            "ch": np.ascontiguousarray(ch, np.float32),
            "wqT": wqT, "wqrT": wqrT, "wkT": wkT, "wkrT": wkrT,
            "wvT": wvT, "woT": woT, "bo4": bo4,
            "cosq": np.ascontiguousarray(cos_tab[:, ts]),
            "sinq": np.ascontiguousarray(sin_tab[:, ts]),
            "cosk": cos_tab, "sink": sin_tab,
        })
    return in_maps


def assemble(results):
    Y = np.empty((B, C, T), np.float32)
    for core in range(NCORES):
        b, j = core // 2, core % 2
        Y[b, :, j * TH:(j + 1) * TH] = results[core]["y"]
    return Y


_CACHE = {}


def _get_program(use_bias):
    key = (USE_F32R, use_bias)
    if key not in _CACHE:
        _CACHE[key] = build_program(USE_F32R, use_bias)
    return _CACHE[key]


def run(trace=False, **inputs):
    use_bias = any(
        np.any(np.asarray(inputs[k])) for k in ("bq", "bk", "bv"))
    nc = _get_program(use_bias)
    in_maps = make_in_maps(use_bias=use_bias, **inputs)
    res = bass_utils.run_bass_kernel_spmd(
        nc, in_maps, core_ids=list(range(NCORES)), trace=trace)
    return assemble(res.results), res


def kernel(**inputs):
    out, _ = run(trace=False, **inputs)
    return out
